# revision 18
# baseline (speedup 1.0000x reference)
"""Trainium2 Bass kernel for the pairwise-MLP adjacency module.

Computes out[b,i,j] = softmax_j( MLP(|v[b,i,:] - v[b,j,:]|) ) where the MLP is
128 -> 64 (leaky 0.1) -> 32 (leaky 0.1) -> 1, implemented as 1x1 convs in the
reference.

Sharding: 8 cores, 2 cores per batch element b (B=4); each core computes 256
of the 512 softmax rows for its b. Weights are replicated, packed host-side.

Per-core dataflow (all matmuls bf16, PSUM f32):
  - DVE: phi_i = |VT - v_i| as one fused tensor_scalar (sub + abs_max) per row.
  - PE L1: W1T (128x64) per row, two rows share a (128,1024) PSUM tile
    (partition halves via tensor-engine column tiling, free halves).
  - ACT: leaky-relu + bias fused into every PSUM->SBUF move (Lrelu, alpha=0.1).
  - PE L2: block-diag2(W2T) processes 2 rows per matmul -> 4 rows per PSUM tile.
  - PE L3: 8 shifted zero-padded copies of block-diag4(W3T) accumulate logits
    for 128 rows *densely* into one PSUM bank (4 col-groups x 8 shifts).
  - ACT: softmax via single Exp with fused row-sum (accum_out); max-subtraction
    is skipped (logits are O(5), fine in f32); DVE reciprocal + scale.
  - b3 is dropped entirely: softmax is shift-invariant.
"""

import sys

for _p in ("/opt/trn_rl_repo",):
    if _p not in sys.path:
        sys.path.insert(0, _p)

from contextlib import ExitStack

import numpy as np
import ml_dtypes

import concourse.bass as bass
import concourse.bacc as bacc
import concourse.tile as tile
from concourse import mybir
from concourse.bass_utils import run_bass_kernel_spmd

BF16 = ml_dtypes.bfloat16

B, N, D = 4, 512, 128
H1, H2 = 64, 32
SLOPE = 0.1
NCORES = 8
IPC = B * N // NCORES      # 256 rows per core
NBATCH = IPC // 128        # 2 softmax batches of 128 rows


def build_nc():
    f32 = mybir.dt.float32
    bf = mybir.dt.bfloat16
    nc = bacc.Bacc("TRN2", target_bir_lowering=False, debug=False)

    vt = nc.dram_tensor("vt", [D, N], bf, kind="ExternalInput").ap()
    vtq = nc.dram_tensor("vtq", [D, IPC], f32, kind="ExternalInput").ap()
    w1t = nc.dram_tensor("w1t", [D, H1], bf, kind="ExternalInput").ap()
    m1t = nc.dram_tensor("m1t", [D, H1], bf, kind="ExternalInput").ap()
    w2bd = nc.dram_tensor("w2bd", [2 * H1, 2 * H2], bf, kind="ExternalInput").ap()
    w3v = nc.dram_tensor("w3v", [128, 8 * H2], bf, kind="ExternalInput").ap()
    b1s = nc.dram_tensor("b1s", [128, 1], f32, kind="ExternalInput").ap()
    b2s = nc.dram_tensor("b2s", [128, 1], f32, kind="ExternalInput").ap()
    outd = nc.dram_tensor("out", [IPC, N], f32, kind="ExternalOutput").ap()

    LR = mybir.ActivationFunctionType.Prelu  # parametric relu: reads alpha (HW-verified)
    EXP = mybir.ActivationFunctionType.Exp
    SUB = mybir.AluOpType.subtract
    MAX = mybir.AluOpType.max
    MIN = mybir.AluOpType.min

    with tile.TileContext(nc) as tc, ExitStack() as ctx:
        singles = ctx.enter_context(tc.tile_pool(name="singles", bufs=1))
        phip = ctx.enter_context(tc.tile_pool(name="phip", bufs=8))
        h1p = ctx.enter_context(tc.tile_pool(name="h1p", bufs=3))
        h2p = ctx.enter_context(tc.tile_pool(name="h2p", bufs=3))
        p1p = ctx.enter_context(tc.tile_pool(name="p1p", bufs=2, space="PSUM"))
        p2p = ctx.enter_context(tc.tile_pool(name="p2p", bufs=2, space="PSUM"))
        lgp = ctx.enter_context(tc.tile_pool(name="lgp", bufs=2, space="PSUM"))
        postp = ctx.enter_context(tc.tile_pool(name="postp", bufs=2))

        vt_sb = singles.tile([D, N], bf)
        nc.sync.dma_start(out=vt_sb, in_=vt)
        vtq_sb = singles.tile([D, IPC], f32)
        nc.sync.dma_start(out=vtq_sb, in_=vtq)
        w1t_sb = singles.tile([D, H1], bf)
        nc.sync.dma_start(out=w1t_sb, in_=w1t)
        m1t_sb = singles.tile([D, H1], bf)
        nc.sync.dma_start(out=m1t_sb, in_=m1t)
        w2_sb = singles.tile([2 * H1, 2 * H2], bf)
        nc.sync.dma_start(out=w2_sb, in_=w2bd)
        w3_sb = singles.tile([128, 8 * H2], bf)
        nc.sync.dma_start(out=w3_sb, in_=w3v)
        b1_sb = singles.tile([128, 1], f32)
        nc.sync.dma_start(out=b1_sb, in_=b1s)
        b2_sb = singles.tile([128, 1], f32)
        nc.sync.dma_start(out=b2_sb, in_=b2s)

        logits = []
        for ib in range(NBATCH):
            lg = lgp.tile([128, N], f32, tag="lg")
            logits.append(lg)
            for q in range(32):
                c0, dsh = divmod(q, 8)
                i0 = ib * 128 + 4 * q
                # |d| = relu(d) - min(d, 0): two 4x-mode tensor_scalars, and
                # L1 accumulates W1T@phiP + (-W1T)@phiM in PSUM.
                phps = []
                phms = []
                for k in range(4):
                    vq = vtq_sb[:, i0 + k : i0 + k + 1]
                    php = phip.tile([D, N], bf, tag="phip")
                    nc.vector.tensor_scalar(
                        out=php, in0=vt_sb, scalar1=vq, scalar2=0.0,
                        op0=SUB, op1=MAX,
                    )
                    phm = phip.tile([D, N], bf, tag="phim")
                    nc.vector.tensor_scalar(
                        out=phm, in0=vt_sb, scalar1=vq, scalar2=0.0,
                        op0=SUB, op1=MIN,
                    )
                    phps.append(php)
                    phms.append(phm)
                p1 = p1p.tile([128, 2 * N], f32, tag="p1")
                for k in range(4):
                    reg = p1[64 * (k % 2) : 64 * (k % 2) + 64,
                             N * (k // 2) : N * (k // 2) + N]
                    nc.tensor.matmul(reg, w1t_sb, phps[k], start=True, stop=False)
                    nc.tensor.matmul(reg, m1t_sb, phms[k], start=False, stop=True)
                h1 = h1p.tile([128, 2 * N], bf, tag="h1")
                nc.scalar.activation(
                    out=h1, in_=p1, func=LR, bias=b1_sb, scale=1.0, alpha=SLOPE
                )
                p2 = p2p.tile([128, N], f32, tag="p2")
                nc.tensor.matmul(p2[0:64, :], w2_sb, h1[:, 0:N], start=True, stop=True)
                nc.tensor.matmul(
                    p2[64:128, :], w2_sb, h1[:, N : 2 * N], start=True, stop=True
                )
                h2 = h2p.tile([128, N], bf, tag="h2")
                nc.scalar.activation(
                    out=h2, in_=p2, func=LR, bias=b2_sb, scale=1.0, alpha=SLOPE
                )
                nc.tensor.matmul(
                    lg[32 * c0 : 32 * c0 + 32, :],
                    w3_sb[:, 32 * dsh : 32 * dsh + 32],
                    h2,
                    start=(dsh == 0),
                    stop=(dsh == 7),
                    tile_position=(0, 32 * c0),
                )
        for ib in range(NBATCH):
            lg = logits[ib]
            expo = postp.tile([128, N], f32, tag="expo")
            sums = postp.tile([128, 1], f32, tag="sums")
            nc.scalar.activation(out=expo, in_=lg, func=EXP, accum_out=sums)
            rs = postp.tile([128, 1], f32, tag="rs")
            nc.vector.reciprocal(rs, sums)
            res = postp.tile([128, N], f32, tag="res")
            nc.vector.tensor_scalar_mul(out=res, in0=expo, scalar1=rs)
            # chain-major L3 ordering makes partition p == output row p
            nc.sync.dma_start(out=outd[ib * 128 : (ib + 1) * 128, :], in_=res)
    return nc


def make_in_maps(v, W1, b1, W2, b2, W3):
    w1t = np.ascontiguousarray(W1.T).astype(BF16)                    # (128, 64)
    m1t = np.ascontiguousarray(-W1.T).astype(BF16)
    w2bd = np.zeros((2 * H1, 2 * H2), np.float32)
    w2bd[0:H1, 0:H2] = W2.T
    w2bd[H1 : 2 * H1, H2 : 2 * H2] = W2.T
    w2bd = w2bd.astype(BF16)
    w3v = np.zeros((128, 8 * H2), np.float32)
    for dsh in range(8):
        for k in range(4):
            w3v[32 * k : 32 * k + 32, 32 * dsh + 4 * dsh + k] = W3[0, :]
    w3v = w3v.astype(BF16)
    b1s = np.concatenate([b1, b1]).reshape(128, 1).astype(np.float32)
    b2s = np.tile(b2, 4).reshape(128, 1).astype(np.float32)

    vts = v.transpose(0, 2, 1)  # (B, D, N)
    in_maps = []
    for c in range(NCORES):
        b, io = c // 2, IPC * (c % 2)
        vt_c = np.ascontiguousarray(vts[b]).astype(BF16)
        in_maps.append(
            {
                "vt": vt_c,
                "vtq": np.ascontiguousarray(
                    vt_c[:, io : io + IPC].astype(np.float32)
                ),
                "w1t": w1t,
                "m1t": m1t,
                "w2bd": w2bd,
                "w3v": w3v,
                "b1s": b1s,
                "b2s": b2s,
            }
        )
    return in_maps


_NC_CACHE = [None]


def get_nc():
    if _NC_CACHE[0] is None:
        nc = build_nc()
        nc.finalize()  # runs the Bacc lowering passes (reg alloc, sem split)
        _NC_CACHE[0] = nc
    return _NC_CACHE[0]


def run(inputs, trace=False, **kw):
    nc = get_nc()
    in_maps = make_in_maps(
        inputs["v"], inputs["W1"], inputs["b1"], inputs["W2"], inputs["b2"],
        inputs["W3"],
    )
    res = run_bass_kernel_spmd(nc, in_maps, list(range(NCORES)), trace=trace, **kw)
    out = np.empty((B, N, N), np.float32)
    for c in range(NCORES):
        b, io = c // 2, IPC * (c % 2)
        out[b, io : io + IPC, :] = np.asarray(res.results[c]["out"], np.float32)
    return out, res


def kernel(**inputs):
    out, _ = run(inputs, trace=False)
    return out
